# revision 7
# baseline (speedup 1.0000x reference)
"""BertEmbedding (scalar-mix + ragged mean-pool + projection) on 8 TRN2 cores.

Full-input contract: kernel(**inputs) takes the unsharded numpy inputs and
returns the full [32, 256, 400] f32 output. Internally: data-parallel over
batch (4 examples per core), proj_w replicated (pre-transposed on host).

Structural choices (v5):
  - Ragged bound: positions p >= sum(bert_lens[b]) fall in the reference's
    overflow bucket and contribute nothing, so only T_pad = roundup-to-4 of
    max_b sum(lens[b]) subword rows are shipped/loaded per example
    (~280 of 512 for the spec's length distribution).
  - bf16 hidden states: the rel-err tolerance (2e-2) admits bf16 for the
    bandwidth-bound hidden tensor; the host ships hid pre-cast to bf16
    (the same rounding the on-device cast DMA would apply), halving HBM
    traffic of the dominant load. Total error stays ~5e-3.
  - Single-block layout: positions are relabeled p = 4*part + q
    (q in 0..3), so each (example, layer) is ONE DMA instruction of 6KB
    contiguous per-partition runs (DMA here is bound by per-run overhead
    and per-instruction SWDGE descgen, not bytes).
  - Equal-mix fast path: when all mix_weights entries are equal (softmax
    exactly uniform, as in the spec's fill=zeros), sum_l w_l*hid_l =
    w_0 * sum_l hid_l: three bf16 tensor-tensor adds on DVE (2x rate),
    with w_0 = gamma*softmax[0] folded into the output's per-partition
    ACT scale together with 1/cnt (both f32, no precision loss).
  - Pooling matmul: pooledT[h, j] += mix_q^T @ M_q with the exact 0/1
    membership M (bf16) as rhs and mixed chunks as PE weights,
    accumulating the 4 position chunks into per-h-slice PSUM banks.
  - Projection in bf16 with the combined w0/cnt scale on the PSUM copy.
  - General-weights fallback: DVE premix with per-layer scalar weights
    (tensor_scalar + 3 scalar_tensor_tensor passes), then the same
    pooling; the output scale is then 1/cnt only.

Membership build, scans, softmax and all scales run in exact f32.

Input-spec property relied on (declared in the problem spec):
  - bert_mask fill=ones -> positions' mask cumsum is the position index.
"""

import numpy as np

NL, B, SW, H = 4, 32, 512, 768
SL, NOUT = 256, 400
NCORES = 8
BPC = B // NCORES  # examples per core
HC = H // 128      # hidden chunks
JC = SL // 128     # word chunks
NQ = 4             # position interleave (6KB bf16 runs)

_NC_CACHE = {}
LAST_RESULT = None  # BassKernelResults of the last run (for profiling)


def _build_nc(T, eq):
    """Per-core program: padded position bound T (multiple of 4, <= 512),
    equal-weights flag."""
    import concourse.bacc as bacc
    import concourse.tile as tile
    from concourse import mybir

    f32 = mybir.dt.float32
    f32r = mybir.dt.float32r
    bf16 = mybir.dt.bfloat16
    i32 = mybir.dt.int32
    Alu = mybir.AluOpType
    Act = mybir.ActivationFunctionType
    Axis = mybir.AxisListType

    assert T % NQ == 0 and 0 < T <= SW
    PT = T // NQ               # partitions holding positions

    nc = bacc.Bacc(None)
    hid = nc.dram_tensor("hid", [NL, BPC, T, H], bf16, kind="ExternalInput")
    lens = nc.dram_tensor("lens", [BPC, SL], i32, kind="ExternalInput")
    mw = nc.dram_tensor("mw", [1, NL], f32, kind="ExternalInput")
    gam = nc.dram_tensor("gam", [1, 1], f32, kind="ExternalInput")
    projT = nc.dram_tensor("projT", [H, NOUT], f32, kind="ExternalInput")
    sel = nc.dram_tensor("sel", [BPC, BPC * 128], f32, kind="ExternalInput")
    out = nc.dram_tensor("out", [BPC, SL, NOUT], f32, kind="ExternalOutput")

    with tile.TileContext(nc) as tc:
        with (
            tc.tile_pool(name="const", bufs=1) as const,
            tc.tile_pool(name="small", bufs=1) as small,
            tc.tile_pool(name="h", bufs=6 if eq else 8) as hpool,
            tc.tile_pool(name="acc", bufs=4) as accpool,
            tc.tile_pool(name="mtmp", bufs=2) as mpool,
            tc.tile_pool(name="Mm", bufs=1) as Mpool,
            tc.tile_pool(name="se", bufs=2) as sepool,
            tc.tile_pool(name="pt", bufs=2) as ptpool,
            tc.tile_pool(name="osb", bufs=2) as opool,
            tc.tile_pool(name="psb", bufs=1, space="PSUM") as ps_b,
            tc.tile_pool(name="psp", bufs=1, space="PSUM") as ps_p,
            tc.tile_pool(name="pso", bufs=1, space="PSUM") as ps_o,
        ):
            # ---- constants ----
            ones_f1 = const.tile([1, 128], f32)
            nc.vector.memset(ones_f1[:], 1.0)
            # one-hot selector (host constant): sel[q, b*128+m] = (q == b);
            # sel_b.T @ rows[BPC, N] broadcasts rows[b] across 128 partitions
            sel_f = const.tile([BPC, BPC * 128], f32)
            nc.sync.dma_start(sel_f[:], sel[:])
            sel_sb = const.tile([BPC, BPC * 128], f32r)
            nc.vector.tensor_copy(sel_sb[:], sel_f[:])

            # ---- lens rows first: they gate the ends/starts scan ----
            lens_i = small.tile([BPC, SL], i32)
            nc.sync.dma_start(lens_i[:], lens[:])

            # ---- lens: ends/starts rows (f32r) ----
            lensf = small.tile([BPC, SL], f32)
            nc.vector.tensor_copy(lensf[:], lens_i[:])
            ends_r = small.tile([BPC, SL], f32r)
            nc.vector.tensor_tensor_scan(out=ends_r[:], data0=lensf[:], data1=lensf[:], initial=0.0, op0=Alu.add, op1=Alu.bypass)
            starts_r = small.tile([BPC, SL], f32r)
            nc.vector.tensor_sub(starts_r[:], ends_r[:], lensf[:])

            # ---- softmax(mix_weights) * gamma, broadcast to [128, NL] ----
            mw_sb = small.tile([1, NL], f32)
            nc.sync.dma_start(mw_sb[:], mw[:])
            gam_sb = small.tile([1, 1], f32)
            nc.sync.dma_start(gam_sb[:], gam[:])
            mmax = small.tile([1, 1], f32)
            nc.vector.tensor_reduce(out=mmax[:], in_=mw_sb[:], axis=Axis.X, op=Alu.max)
            nmax = small.tile([1, 1], f32)
            nc.vector.tensor_scalar(out=nmax[:], in0=mmax[:], scalar1=-1.0, scalar2=None, op0=Alu.mult)
            mexp = small.tile([1, NL], f32)
            nc.scalar.activation(out=mexp[:], in_=mw_sb[:], func=Act.Exp, bias=nmax[:], scale=1.0)
            msum = small.tile([1, 1], f32)
            nc.vector.tensor_reduce(out=msum[:], in_=mexp[:], axis=Axis.X, op=Alu.add)
            mrec = small.tile([1, 1], f32)
            nc.vector.reciprocal(out=mrec[:], in_=msum[:])
            w_row = small.tile([1, NL], f32)
            nc.vector.tensor_scalar(out=w_row[:], in0=mexp[:], scalar1=mrec[:], scalar2=gam_sb[:], op0=Alu.mult, op1=Alu.mult)
            ps_w = ps_o.tile([128, NL], f32, tag="po")
            nc.tensor.matmul(out=ps_w[:], lhsT=ones_f1[:], rhs=w_row[:], start=True, stop=True)
            w_sb = small.tile([128, NL], f32)
            nc.scalar.copy(w_sb[:], ps_w[:])

            # ---- per-position ids: cs[part, q] = 4*part + q + 1 ----
            cs_i = small.tile([128, NQ], i32)
            nc.gpsimd.iota(cs_i[:], pattern=[[1, NQ]], base=1, channel_multiplier=NQ)
            cs_sb = small.tile([128, NQ], f32)
            nc.vector.tensor_copy(cs_sb[:], cs_i[:])

            # ---- membership matrices for ALL examples up front ----
            # M = exact 0/1 (bf16); any global scale is applied at the end
            Mts = []
            for b in range(BPC):
                ps_se = ps_b.tile([128, 2 * SL], f32, tag="se")
                sel_b = sel_sb[:, b * 128:(b + 1) * 128]
                nc.tensor.matmul(out=ps_se[:, 0:SL], lhsT=sel_b, rhs=starts_r[:], start=True, stop=True)
                nc.tensor.matmul(out=ps_se[:, SL:2 * SL], lhsT=sel_b, rhs=ends_r[:], start=True, stop=True)
                se_sb = sepool.tile([128, 2 * SL], f32, tag="sesb")
                nc.scalar.copy(se_sb[:], ps_se[:])

                Mt = Mpool.tile([128, NQ, SL], bf16, tag=f"M{b}", name=f"M{b}")
                for q in range(NQ):
                    csc = cs_sb[:, q:q + 1]
                    m2 = mpool.tile([128, SL], f32, tag="m2")
                    nc.vector.tensor_scalar(
                        out=m2[:], in0=se_sb[:, SL:2 * SL], scalar1=csc,
                        scalar2=None, op0=Alu.is_ge)
                    nc.vector.scalar_tensor_tensor(
                        out=Mt[:, q, :], in0=se_sb[:, 0:SL], scalar=csc,
                        in1=m2[:], op0=Alu.is_lt, op1=Alu.mult)
                Mts.append(Mt)

            # ---- hidden loads: ONE DMA per (example, layer) ----
            hts_all = [[] for _ in range(BPC)]

            def emit_hid(b, l):
                ht = hpool.tile([128, NQ, H], bf16, tag="h", name=f"h{b}_{l}")
                nc.gpsimd.dma_start(
                    ht[0:PT, :, :],
                    hid[l, b, :, :].rearrange("(p q) d -> p q d", q=NQ))
                hts_all[b].append(ht)

            def emit_tail_loads():
                # small loads on HWDGE so they don't cost SWDGE descgen
                projT_f = const.tile([128, HC, NOUT], f32)
                nc.sync.dma_start(projT_f[:], projT.rearrange("(i p) o -> p i o", p=128))
                projT_sb = const.tile([128, HC, NOUT], bf16)
                nc.vector.tensor_copy(projT_sb[:], projT_f[:])
                lensc_i = small.tile([128, JC, BPC], i32)
                for jh in range(JC):
                    nc.sync.dma_start(lensc_i[:, jh, :], lens[:, jh * 128:(jh + 1) * 128].rearrange("b p -> p b"))
                lensc_f = small.tile([128, JC, BPC], f32)
                nc.vector.tensor_copy(lensc_f[:], lensc_i[:])
                lensc_m = small.tile([128, JC, BPC], f32)
                nc.vector.tensor_scalar_max(lensc_m[:], lensc_f[:], 1.0)
                invcnt = small.tile([128, JC, BPC], f32)
                nc.vector.reciprocal(out=invcnt[:], in_=lensc_m[:])
                osc = small.tile([128, JC, BPC], f32)
                if eq:
                    # combined output scale: w0 * 1/cnt (both exact f32)
                    nc.vector.tensor_scalar(out=osc[:], in0=invcnt[:], scalar1=w_sb[:, 0:1], scalar2=None, op0=Alu.mult)
                else:
                    nc.vector.tensor_copy(osc[:], invcnt[:])
                return projT_sb, osc

            for b in range(BPC):
                for l in range(NL):
                    emit_hid(b, l)
                    if b == 0 and l == 0:
                        projT_sb, osc = emit_tail_loads()

            # ---- per-example compute pipeline ----
            for b in range(BPC):
                Mt = Mts[b]
                hts = hts_all[b]
                if eq:
                    # unweighted layer sum (scale folded into output)
                    s01 = accpool.tile([128, NQ, H], bf16, tag="s01")
                    nc.vector.tensor_add(s01[0:PT], hts[0][0:PT], hts[1][0:PT])
                    s23 = accpool.tile([128, NQ, H], bf16, tag="s23")
                    nc.vector.tensor_add(s23[0:PT], hts[2][0:PT], hts[3][0:PT])
                    mm = accpool.tile([128, NQ, H], bf16, tag="mm")
                    nc.vector.tensor_add(mm[0:PT], s01[0:PT], s23[0:PT])
                else:
                    # premix: mixed = sum_l w[l] * hid[l] (DVE)
                    prev = None
                    for l in range(NL):
                        dst = accpool.tile([128, NQ, H], bf16, tag=f"px{l}", name=f"px{l}")
                        wl = w_sb[0:PT, l:l + 1]
                        if l == 0:
                            nc.vector.tensor_scalar(
                                out=dst[0:PT], in0=hts[l][0:PT],
                                scalar1=wl, scalar2=None, op0=Alu.mult)
                        else:
                            nc.vector.scalar_tensor_tensor(
                                out=dst[0:PT], in0=hts[l][0:PT],
                                scalar=wl, in1=prev[0:PT], op0=Alu.mult, op1=Alu.add)
                        prev = dst
                    mm = prev

                # ---- ragged mean-pool: pooledT[h, j] += mix_q^T @ M_q ----
                # one PSUM bank per h-slice: interleaved accumulation groups
                # are only correct across different banks (HW-verified)
                pps = []
                for i in range(HC):
                    pp_i = ps_p.tile([128, SL], f32, tag=f"pp{i}", name=f"pp{i}")
                    pps.append(pp_i)
                for q in range(NQ):
                    for i in range(HC):
                        nc.tensor.matmul(
                            out=pps[i][:],
                            lhsT=mm[0:PT, q, i * 128:(i + 1) * 128],
                            rhs=Mt[0:PT, q, :],
                            start=(q == 0),
                            stop=(q == NQ - 1),
                            skip_group_check=True,
                        )
                ptsb = ptpool.tile([128, HC, SL], bf16, tag="pt")
                for i in range(HC):
                    nc.scalar.copy(ptsb[:, i, :], pps[i][:])

                # projection (bf16) + combined scale on the PSUM->SBUF copy
                osb = opool.tile([128, JC, NOUT], f32, tag="o")
                for jh in range(JC):
                    po = ps_o.tile([128, NOUT], f32, tag="po")
                    for i in range(HC):
                        nc.tensor.matmul(
                            out=po[:],
                            lhsT=ptsb[:, i, jh * 128:(jh + 1) * 128],
                            rhs=projT_sb[:, i, :],
                            start=(i == 0),
                            stop=(i == HC - 1),
                        )
                    nc.scalar.activation(out=osb[:, jh, :], in_=po[:], func=Act.Copy, scale=osc[:, jh, b:b + 1])
                nc.scalar.dma_start(out[b].rearrange("(jh p) o -> p jh o", p=128), osb[:])

    nc.finalize()
    return nc


def _get_nc(key):
    if key not in _NC_CACHE:
        _NC_CACHE[key] = _build_nc(*key)
    return _NC_CACHE[key]


def kernel(subwords=None, bert_lens=None, bert_mask=None, hidden_states=None,
           mix_weights=None, gamma=None, proj_w=None, **_ignored):
    global LAST_RESULT
    import os
    import ml_dtypes
    from concourse.bass_utils import run_bass_kernel_spmd

    hs = np.asarray(hidden_states, dtype=np.float32)
    lens_np = np.asarray(bert_lens).astype(np.int32)
    mw_np = np.asarray(mix_weights, dtype=np.float32).reshape(1, NL)
    gam_np = np.asarray(gamma, dtype=np.float32).reshape(1, 1)
    projT_np = np.ascontiguousarray(np.asarray(proj_w, dtype=np.float32).T)
    sel_np = np.zeros((BPC, BPC * 128), dtype=np.float32)
    for b in range(BPC):
        sel_np[b, b * 128:(b + 1) * 128] = 1.0

    # program specialization from the runtime inputs (cached per key):
    # ragged position bound (padded to the interleave), equal-weights path
    T = int(min(max(int(lens_np.sum(axis=1).max()), 1), SW))
    T = min(SW, -(-T // NQ) * NQ)
    eq = bool(np.all(mw_np == mw_np.flat[0]))
    nc = _get_nc((T, eq))

    hs_b = np.ascontiguousarray(hs[:, :, :T, :]).astype(ml_dtypes.bfloat16)
    in_maps = []
    for c in range(NCORES):
        sl = slice(c * BPC, (c + 1) * BPC)
        in_maps.append({
            "hid": np.ascontiguousarray(hs_b[:, sl]),
            "lens": np.ascontiguousarray(lens_np[sl]),
            "mw": mw_np,
            "gam": gam_np,
            "projT": projT_np,
            "sel": sel_np,
        })

    trace = bool(int(os.environ.get("KERNEL_TRACE", "0")))
    LAST_RESULT = run_bass_kernel_spmd(nc, in_maps, list(range(NCORES)), trace=trace)
    res = LAST_RESULT.results
    return np.concatenate([r["out"] for r in res], axis=0)
